# revision 3
# baseline (speedup 1.0000x reference)
"""DETR attention kernel for 8 Trainium2 NeuronCores.

Problem: B=8, S=1024, D=256, H=8 heads (head_dim=32), fp32.
  hs_pos = hidden_states + object_queries
  q = (hs_pos @ Wq.T + bq) * sqrt(head_dim)   # NOTE: multiplies (DETR quirk)
  k = hs_pos @ Wk.T + bk
  v = hidden_states @ Wv.T + bv
  out = softmax(q k^T) v @ Wo.T + bo

Sharding: pure data parallel — core b computes batch element b. No collectives.

Per-core pipeline:
  - host pre-transposes inputs/weights (layout only, no FLOPs moved to host)
  - PE (fp32r): projections qT,kT [o,s] / v [t,c]; scores per head via K=32
    row-tiled matmuls; probs@V via K=128 col-tiled matmuls; final proj.
  - DVE: rowmax of scores (PSUM), reciprocal of denominators.
  - ACT: exp(scores - max) with fused row-sum (accum_out), PSUM->SBUF
    copybacks with per-partition bias.
  - GPSIMD: hsp = hs+oq add, probs normalization (per-partition scale).
  - DMA xbar: fp16 probs transpose (SBUF->SBUF) for the probs@V contraction.
"""

import sys

if "/opt/trn_rl_repo" not in sys.path:
    sys.path.insert(0, "/opt/trn_rl_repo")

import numpy as np

B, S, D, H, DH = 8, 1024, 256, 8, 32
SCALING = DH**0.5
N_CORES = 8
P = 128
NSB = S // P  # 8 s-blocks
NCH = D // P  # 2 channel chunks

_cached = {}


def _build_kernel():
    import concourse.bass as bass
    import concourse.mybir as mybir
    import concourse.tile as tile
    from concourse import bacc
    from contextlib import ExitStack

    dt = mybir.dt
    f32, f16, f32r = dt.float32, dt.float16, dt.float32r
    AF = mybir.ActivationFunctionType
    AX = mybir.AxisListType
    OP = mybir.AluOpType

    nc = bacc.Bacc("TRN2", target_bir_lowering=False, debug=False,
                   num_devices=N_CORES)

    def din(name, shape, dt_=f32):
        return nc.dram_tensor(name, shape, dt_, kind="ExternalInput").ap()

    # Host-prepared layouts (see _prep_inputs):
    hs_t = din("hs_t", [P, NCH, S], f32r)      # [p, ch, s] = hs[s, ch*128+p]
    oq_t = din("oq_t", [P, NCH, S], f32r)
    wq_t = din("wq_t", [P, NCH, D], f32r)      # [p, ich, o] = (Wq.T*SCALING)[ich*128+p, o]
    wk_t = din("wk_t", [P, NCH, D], f32r)
    wv_t = din("wv_t", [P, NCH, D], f32r)
    wo_t = din("wo_t", [P, NCH, D], f32r)      # Wo.T
    bq_c = din("bq_c", [P, NCH])         # (bq*SCALING) arranged per-partition
    bk_c = din("bk_c", [P, NCH])
    bv_r = din("bv_r", [1, D], f16)
    bo_r = din("bo_r", [1, D], f16)
    out_ap = nc.dram_tensor("out", [P, NSB, D], f32, kind="ExternalOutput").ap()

    with tile.TileContext(nc) as tc, ExitStack() as ctx:
        const = ctx.enter_context(tc.tile_pool(name="const", bufs=1))
        inp = ctx.enter_context(tc.tile_pool(name="inp", bufs=1))
        qk = ctx.enter_context(tc.tile_pool(name="qk", bufs=1))
        probs_pool = ctx.enter_context(tc.tile_pool(name="probs", bufs=2))
        probsT_pool = ctx.enter_context(tc.tile_pool(name="probsT", bufs=1))
        stats = ctx.enter_context(tc.tile_pool(name="stats", bufs=8))
        outp = ctx.enter_context(tc.tile_pool(name="outp", bufs=3))
        ps_big = ctx.enter_context(
            tc.tile_pool(name="ps_big", bufs=3, space="PSUM"))
        ps_small = ctx.enter_context(
            tc.tile_pool(name="ps_small", bufs=2, space="PSUM"))

        # ---- constants / inputs ----
        ones_row = const.tile([1, P], f16)
        nc.gpsimd.memset(ones_row[:], 1.0)

        hsT = inp.tile([P, NCH, S], f32r, tag="hsT")
        oqT = inp.tile([P, NCH, S], f32r, tag="oqT")
        nc.sync.dma_start(hsT[:], hs_t[:])
        nc.sync.dma_start(oqT[:], oq_t[:])

        w_q = const.tile([P, NCH, D], f32r, tag="w_q")
        w_k = const.tile([P, NCH, D], f32r, tag="w_k")
        w_v = const.tile([P, NCH, D], f32r, tag="w_v")
        w_o = const.tile([P, NCH, D], f32r, tag="w_o")
        b_q = const.tile([P, NCH], f32, tag="b_q")
        b_k = const.tile([P, NCH], f32, tag="b_k")
        b_v = const.tile([1, D], f16, tag="b_v")
        b_o = const.tile([1, D], f16, tag="b_o")
        for t_, d_ in [(w_q, wq_t), (w_k, wk_t), (w_v, wv_t), (w_o, wo_t),
                       (b_q, bq_c), (b_k, bk_c), (b_v, bv_r), (b_o, bo_r)]:
            nc.sync.dma_start(t_[:], d_[:])

        # ---- hspT = hsT + oqT (GPSIMD, keep DVE free) ----
        hspT = inp.tile([P, NCH, S], f32r, tag="hspT")
        for ch in range(NCH):
            nc.gpsimd.tensor_tensor(hspT[:, ch], hsT[:, ch], oqT[:, ch], OP.add)

        # ---- projections (fp32r matmuls) ----
        qT = qk.tile([P, NCH, S], f32r, tag="qT")
        kT = qk.tile([P, NCH, S], f32r, tag="kT")
        for w_, b_, dst in [(w_q, b_q, qT), (w_k, b_k, kT)]:
            for och in range(NCH):
                ps = ps_big.tile([P, S], f32, tag="big")
                for ich in range(NCH):
                    for sh in range(2):
                        nc.tensor.matmul(
                            ps[:, sh * 512:(sh + 1) * 512],
                            lhsT=w_[:, ich, och * P:(och + 1) * P],
                            rhs=hspT[:, ich, sh * 512:(sh + 1) * 512],
                            start=(ich == 0), stop=(ich == NCH - 1))
                nc.scalar.activation(dst[:, och], ps[:], AF.Identity,
                                     bias=b_[:, och:och + 1], scale=1.0)

        v_sb = qk.tile([P, NSB, D], f16, tag="v_sb")
        for tb in range(NSB):
            ps = ps_small.tile([P, 512], f32, tag="small")
            for ich in range(NCH):
                nc.tensor.matmul(
                    ps[:, :D],
                    lhsT=hsT[:, ich, tb * P:(tb + 1) * P],
                    rhs=w_v[:, ich],
                    start=(ich == 0), stop=False)
            nc.tensor.matmul(ps[:, :D], lhsT=ones_row,
                             rhs=b_v, start=False, stop=True)
            nc.vector.tensor_copy(v_sb[:, tb], ps[:, :D])

        # ---- attention ----
        attnT = qk.tile([P, NCH, S], f32r, tag="attnT")
        for half in range(2):
            probsT_r = probsT_pool.tile([P, 4, H * NSB, P], f16, tag="pT")
            for sb4 in range(4):
                sb = half * 4 + sb4
                probs_sb = probs_pool.tile([P, H, S], f16, tag="probs")
                negmax = stats.tile([P, H], f32, tag="negmax")
                den = stats.tile([P, H], f32, tag="den")
                invden = stats.tile([P, H], f32, tag="invden")
                for h in range(H):
                    hch, hr = divmod(h, 4)
                    ps_s = ps_big.tile([P, S], f32, tag="big")
                    for th in range(2):
                        nc.tensor.matmul(
                            ps_s[:, th * 512:(th + 1) * 512],
                            lhsT=qT[32 * hr:32 * hr + 32, hch,
                                    sb * P:(sb + 1) * P],
                            rhs=kT[32 * hr:32 * hr + 32, hch,
                                   th * 512:(th + 1) * 512],
                            start=True, stop=True,
                            tile_position=(32 * hr, 0))
                    nc.vector.tensor_reduce(
                        out=negmax[:, h:h + 1], in_=ps_s[:], axis=AX.X,
                        op=OP.max, negate=True)
                    nc.scalar.activation(
                        probs_sb[:, h], ps_s[:], AF.Exp,
                        bias=negmax[:, h:h + 1], scale=1.0,
                        accum_out=den[:, h:h + 1])
                nc.vector.reciprocal(invden[:], den[:])
                for h in range(H):
                    nc.gpsimd.tensor_scalar_mul(
                        probs_sb[:, h], probs_sb[:, h], invden[:, h:h + 1])
                nc.sync.dma_start_transpose(out=probsT_r[:, sb4],
                                            in_=probs_sb[:])

            # probs @ V for this half (4 s-blocks batched, col-tiled heads)
            for grp in range(2):
                ps_a = ps_small.tile([P, 512], f32, tag="small")
                for tcn in range(NSB):
                    for hh in range(4):
                        h = grp * 4 + hh
                        nc.tensor.matmul(
                            ps_a[32 * hh:32 * hh + 32, :],
                            lhsT=v_sb[:, tcn, 32 * h:32 * h + 32],
                            rhs=probsT_r[:, :, h * NSB + tcn, :],
                            start=(tcn == 0), stop=(tcn == NSB - 1),
                            tile_position=(0, 32 * hh))
                nc.scalar.copy(attnT[:, grp, half * 512:(half + 1) * 512],
                               ps_a[:])

            # final projection for the 4 finished s-blocks
            for sb4 in range(4):
                sb = half * 4 + sb4
                ps_f = ps_small.tile([P, 512], f32, tag="small")
                for ch in range(NCH):
                    nc.tensor.matmul(
                        ps_f[:, :D],
                        lhsT=attnT[:, ch, sb * P:(sb + 1) * P],
                        rhs=w_o[:, ch],
                        start=(ch == 0), stop=False)
                nc.tensor.matmul(ps_f[:, :D], lhsT=ones_row,
                                 rhs=b_o, start=False, stop=True)
                out_t = outp.tile([P, D], f32, tag="out")
                nc.scalar.copy(out_t[:], ps_f[:, :D])
                nc.sync.dma_start(out_ap[:, sb, :], out_t[:])

    nc.compile()
    return nc


def _prep_inputs(hidden_states, object_queries, Wq, bq, Wk, bk, Wv, bv, Wo, bo):
    """Per-core input maps. Layout transforms only (transpose/reshape/scale-fold)."""
    def colmajor(x):  # [S, D] -> [p, ch, s]
        return np.ascontiguousarray(
            x.T.reshape(NCH, P, S).transpose(1, 0, 2))

    def wprep(w, scale=1.0):  # nn.Linear W [out,in] -> W.T [p, ich, o]
        wt = (w.T * scale).astype(np.float32)
        return np.ascontiguousarray(wt.reshape(NCH, P, D).transpose(1, 0, 2))

    def bcol(b, scale=1.0):  # [D] -> [p, ch]
        return np.ascontiguousarray(
            (b * scale).astype(np.float32).reshape(NCH, P).T)

    shared = {
        "wq_t": wprep(Wq, SCALING), "wk_t": wprep(Wk), "wv_t": wprep(Wv),
        "wo_t": wprep(Wo),
        "bq_c": bcol(bq, SCALING), "bk_c": bcol(bk),
        "bv_r": np.ascontiguousarray(bv.astype(np.float16)[None, :]),
        "bo_r": np.ascontiguousarray(bo.astype(np.float16)[None, :]),
    }
    in_maps = []
    for b in range(B):
        m = dict(shared)
        m["hs_t"] = colmajor(hidden_states[b])
        m["oq_t"] = colmajor(object_queries[b])
        in_maps.append(m)
    return in_maps


def kernel(hidden_states, object_queries, Wq, bq, Wk, bk, Wv, bv, Wo, bo,
           _trace=False, _trace_kwargs=None):
    from concourse.bass_utils import run_bass_kernel_spmd

    if "nc" not in _cached:
        _cached["nc"] = _build_kernel()
    nc = _cached["nc"]

    in_maps = _prep_inputs(np.asarray(hidden_states, dtype=np.float32),
                           np.asarray(object_queries, dtype=np.float32),
                           np.asarray(Wq), np.asarray(bq), np.asarray(Wk),
                           np.asarray(bk), np.asarray(Wv), np.asarray(bv),
                           np.asarray(Wo), np.asarray(bo))
    kw = dict(_trace_kwargs or {})
    res = run_bass_kernel_spmd(nc, in_maps, core_ids=list(range(N_CORES)),
                               trace=_trace, **kw)
    out = np.empty((B, S, D), dtype=np.float32)
    for b in range(B):
        o = res.results[b]["out"]  # [p, sb, e]
        out[b] = o.transpose(1, 0, 2).reshape(S, D)
    if _trace:
        kernel._last_results = res
    return out
